# revision 33
# baseline (speedup 1.0000x reference)
"""HFreqC layer kernel for 8 Trainium2 NeuronCores.

The reference op (FFT -> zero centered low-freq band -> IFFT -> real -> relu)
is a fixed real circulant operator along the 728-channel axis followed by
relu. For scale=4 the kept band k in [-182, 181] is a complete residue
system mod 364, so the 728x728 circulant splits exactly into two 364x364
polyphase matmuls sharing one kernel matrix T (verified to 5e-16):
    y_even = relu(x_e/2 - (T  x_o)/2)
    y_odd  = relu(x_o/2 - (T' x_e)/2),   T' = T^T
This HALVES tensor-engine work vs the dense 728x728 matmul.

Layout: pure data parallel over rows; 46208 rows = 361 tiles of 128; cores
0-6 take 45 tiles (+1 zero pad), core 7 takes 46 -> every core runs 5888
rows. The host splits channels into even/odd polyphases, transposes to
channel-major, casts to bf16. Per core, per 512-row chunk:
  - rhs = x^T polyphase k-tiles [128, N<=512] (moving), lhsT = -T/2 tiles
    [128, 128] (stationary, bf16, FWL) -> PSUM [j, rows] over 3 k-passes.
  - VectorE scalar_tensor_tensor adds x/2 of the output polyphase in-place
    in PSUM (the bias is a slice of the already-loaded input - same
    channel-major layout, no extra DMA).
  - ScalarE applies relu+cast-to-bf16 on the PSUM->SBUF copy; bf16 DMA out
    (host de-interleaves polyphases and upcasts).
"""

import numpy as np

C = 728            # channels
H = 364            # polyphase channels (C/2)
KT = 3             # k tiles of 128 per polyphase (pad 364 -> 384)
HP = KT * 128      # 384
TILES = 46         # 128-row tiles per core (45 real + 1 pad on cores 0-6)
ROWS_PAD = TILES * 128             # 5888
N_CORES = 8
ROWS_TOTAL = 32 * 38 * 38          # 46208 = 361 * 128
NCHUNK = 2048      # rows per DMA chunk (4KB per-partition runs)
CHUNKS = [2048] * 2 + [1792]       # 5888 rows; compute in 512-row sub-chunks
HV = 108           # valid partitions in the last k/j tile (364 - 256)

# per-core row ranges: cores 0-6 own 45 tiles, core 7 owns 46
_TILE_START = [i * 45 for i in range(N_CORES)]
_TILE_CNT = [45] * 7 + [46]

_CACHE = {}


def _t_mat() -> np.ndarray:
    """[H, H] f32 circulant T: (T v) = ifft364(t * fft364(v)), t_k =
    sigma_k e^{-2pi i k/728}, sigma = +1 (k<=181) else -1."""
    k = np.arange(H)
    t = np.where(k <= 181, 1.0, -1.0) * np.exp(-2j * np.pi * k / C)
    T = np.real(np.fft.ifft(t[:, None] * np.fft.fft(np.eye(H), axis=0), axis=0))
    return T.astype(np.float32)


def _bf16(a: np.ndarray):
    import ml_dtypes
    return a.astype(ml_dtypes.bfloat16)


def _build_w_host() -> np.ndarray:
    """[128, 2*KT*KT*128] bf16: w[p, mat, u, v, j] = L_mat[u*128+p, v*128+j],
    L_0 = -T^T/2 (even outputs), L_1 = -T/2 (odd outputs), zero padded."""
    T = _t_mat()
    out = np.zeros((128, 2, KT, KT, 128), dtype=np.float32)
    for mat, L in enumerate((-0.5 * T.T, -0.5 * T)):
        Lp = np.zeros((HP, HP), dtype=np.float32)
        Lp[:H, :H] = L
        v4 = Lp.reshape(KT, 128, KT, 128)          # u p v j
        out[:, mat] = v4.transpose(1, 0, 2, 3)     # p u v j
    return _bf16(out.reshape(128, -1))


def _shard_xt(xf: np.ndarray, core: int) -> np.ndarray:
    """[128, 2, KT, ROWS_PAD] bf16: [p, poly, u, row] = x[row, 2*(u*128+p)+poly]."""
    t0, cnt = _TILE_START[core], _TILE_CNT[core]
    xp = np.zeros((ROWS_PAD, HP, 2), dtype=np.float32)
    xp[:cnt * 128, :H, :] = xf[t0 * 128: (t0 + cnt) * 128].reshape(-1, H, 2)
    v = _bf16(xp).reshape(ROWS_PAD, KT, 128, 2)    # row u p poly
    return np.ascontiguousarray(v.transpose(2, 3, 1, 0))  # p poly u row


def _build_nc(repeat: int = 1, loop: int = 0, bias_mode: str = "dve"):
    """loop>1 wraps the sweep in a hardware For_i loop executing it `loop`
    times per dispatch - used only for steady-state timing (the dispatch
    floor under axon is ~3-10ms, far above one sweep)."""
    import concourse.mybir as mybir
    import concourse.tile as tile
    from concourse import bacc

    fp32 = mybir.dt.float32
    bf16 = mybir.dt.bfloat16
    Alu = mybir.AluOpType

    nc = bacc.Bacc("TRN2", target_bir_lowering=False)
    x_d = nc.dram_tensor("x", [128, 2, KT, ROWS_PAD], bf16,
                         kind="ExternalInput").ap()
    w_d = nc.dram_tensor("w", [128, 2 * KT * KT * 128], bf16,
                         kind="ExternalInput").ap()
    y_d = nc.dram_tensor("y", [128, 2, KT, ROWS_PAD], bf16,
                         kind="ExternalOutput").ap()

    with tile.TileContext(nc) as tc:
        with (
            tc.tile_pool(name="wpool", bufs=1) as wpool,
            tc.tile_pool(name="io", bufs=3) as io,
            tc.tile_pool(name="ps2p", bufs=2, space="PSUM") as ps2p,
            tc.tile_pool(name="ps1p", bufs=4, space="PSUM") as ps1p,
        ):
            w_sb = wpool.tile([128, 2 * KT * KT * 128], bf16)
            nc.sync.dma_start(out=w_sb, in_=w_d)

            def wt(mat, u, v):
                o = ((mat * KT + u) * KT + v) * 128
                return w_sb[:, o: o + 128]

            def sweep():
                for _r in range(repeat):
                    r0 = 0
                    for N in CHUNKS:
                        xt = io.tile([128, 2 * KT * NCHUNK], bf16, tag="xt")
                        xt_v = xt.rearrange("p (poly u n) -> p poly u n",
                                            poly=2, u=KT, n=NCHUNK)
                        nc.sync.dma_start(
                            out=xt.rearrange("p (c n) -> p c n",
                                             c=2 * KT, n=NCHUNK)[:, :, :N],
                            in_=x_d.rearrange("p poly u n -> p (poly u) n")
                            [:, :, r0:r0 + N])
                        ysb = io.tile([128, 2 * KT * NCHUNK], bf16, tag="y")
                        ysb_f = ysb.rearrange("p (poly v n) -> p poly v n",
                                              poly=2, v=KT, n=NCHUNK)
                        for s0 in range(0, N, 512):
                            Ns = min(512, N - s0)
                            se = s0 + Ns
                            ysb_v = ysb_f[:, :, :, s0:se].rearrange(
                                "p poly v n -> p poly v n")
                            for po in range(2):  # output polyphase
                                rp = 1 - po     # rhs (input) polyphase
                                ps2 = ps2p.tile([128, 2 * 512], fp32, tag="ps2")
                                ps1 = ps1p.tile([128, 512], fp32, tag="ps1")
                                ps2_v = ps2.rearrange("p (v n) -> p v n",
                                                      v=2, n=512)
                                for v in range(KT):
                                    dst = ps2_v[:, v, :Ns] if v < 2 else ps1[:, :Ns]
                                    for u in range(KT):
                                        nc.tensor.matmul(
                                            dst,
                                            lhsT=wt(po, u, v),
                                            rhs=xt_v[:, rp, u, s0:se],
                                            start=(u == 0),
                                            stop=(u == KT - 1),
                                        )
                                # ps += x_po/2   (the identity/bias term)
                                if bias_mode == "dve":
                                    nc.vector.scalar_tensor_tensor(
                                        ps2_v[:, :, :Ns],
                                        xt_v[:, po, 0:2, s0:se],
                                        0.5,
                                        ps2_v[:, :, :Ns],
                                        Alu.mult,
                                        Alu.add,
                                    )
                                    nc.vector.scalar_tensor_tensor(
                                        ps1[:, :Ns],
                                        xt_v[:, po, 2, s0:se],
                                        0.5,
                                        ps1[:, :Ns],
                                        Alu.mult,
                                        Alu.add,
                                    )
                                nc.scalar.activation(
                                    ysb_f[:, po, 0:2, s0:se],
                                    ps2_v[:, :, :Ns],
                                    mybir.ActivationFunctionType.Relu,
                                )
                                nc.scalar.activation(
                                    ysb_f[:, po, 2, s0:se],
                                    ps1[:, :Ns],
                                    mybir.ActivationFunctionType.Relu,
                                )
                        nc.sync.dma_start(
                            out=y_d.rearrange("p poly v n -> p (poly v) n")
                            [:, :, r0:r0 + N],
                            in_=ysb.rearrange("p (c n) -> p c n",
                                              c=2 * KT, n=NCHUNK)[:, :, :N])
                        r0 += N

            if loop > 1:
                hint = (mybir.EngineType.PE, mybir.EngineType.Activation,
                        mybir.EngineType.SP, mybir.EngineType.DVE)
                with tc.For_i(0, loop, 1, hint_engines=hint):
                    sweep()
            else:
                sweep()
    nc.compile()
    return nc


def _make_in_maps(x: np.ndarray, scale: int):
    xf = np.asarray(x, dtype=np.float32).reshape(-1, C)
    W = _build_w_host()
    return [{"x": _shard_xt(xf, i), "w": W} for i in range(N_CORES)]


def _host_reference(x: np.ndarray, scale: int) -> np.ndarray:
    """Fallback for scale != 4 (harness always passes 4)."""
    c = x.shape[-1]
    m = np.ones(c)
    m[c // 2 - c // scale: c // 2 + c // scale] = 0
    m = np.fft.ifftshift(m)
    X = np.fft.fft(x.astype(np.complex64), axis=-1)
    y = np.real(np.fft.ifft(m * X, axis=-1)).astype(np.float32)
    return np.maximum(y, 0.0)


def kernel(x: np.ndarray, scale) -> np.ndarray:
    import sys
    if "/opt/trn_rl_repo" not in sys.path:
        sys.path.insert(0, "/opt/trn_rl_repo")
    from concourse.bass_utils import run_bass_kernel_spmd

    scale = int(np.asarray(scale))
    x = np.asarray(x, dtype=np.float32)
    orig_shape = x.shape
    if scale != 4:
        return _host_reference(x, scale)

    if "nc" not in _CACHE:
        _CACHE["nc"] = _build_nc()
    nc = _CACHE["nc"]

    in_maps = _make_in_maps(x, scale)
    res = run_bass_kernel_spmd(nc, in_maps, list(range(N_CORES)))
    outs = []
    for i, r in enumerate(res.results):
        yc = np.asarray(r["y"], dtype=np.float32)    # [128, 2, KT, ROWS_PAD]
        yc = yc.transpose(3, 2, 0, 1).reshape(ROWS_PAD, HP, 2)
        yc = yc[:, :H, :].reshape(ROWS_PAD, C)
        outs.append(yc[:_TILE_CNT[i] * 128])
    y = np.concatenate(outs, axis=0).reshape(orig_shape)
    return y.astype(np.float32)
